# revision 7
# baseline (speedup 1.0000x reference)
"""Trainium2 Bass kernel for nn_CustomLoss_90537910600076 (nms_detection).

Computes, for in_signal/ref_signal [2048, 4096] f32:
  [total_loss, cosine_similarity, p2p_loss, mse_loss]  (f32 [4])

Pure data parallel over the batch dim across 8 NeuronCores (256 rows per
core, 2 blocks of 128 partitions). The device computes per-row sufficient
statistics; the host combines them:
  col0 dot    = sum(in*ref)
  col1 na2    = sum(in^2)
  col2 nb2    = sum(ref^2)
  col3 n_in   = #peaks(in, distance=20)
  col4 n_ref  = #peaks(ref, distance=20)
  col5 p2p    = sum((pk10(in) - pk10(ref))^2)

Peak criterion: the reference's strict-local-max AND x >= window-max is,
for tie-free data (random f32 normals; adjacent exact ties have measure
~1e-8 and change the loss by <1e-6 rel), equivalent to plain
x >= max(window) with the window INCLUDING x. So:
  peak_d10(j)  <=>  x[j] >= M19[j]            (M19 = 19-window max)
  peak_d20(j)  <=>  x[j] >= max(M19[j-10], M19[j+10]) =: T39[j]
(the j-10/j+10 windows tile [j-19..j+19]\{j}; excluding x[j] itself is
harmless since the test is >=).

M19 is built once per signal in pair space and written interleaved with
the window EDGE element folded into the final parity levels:
  p[i]  = max(s[2i], s[2i+1])
  P2[i] = max(p[i],  p[i+1])
  P4[i] = max(P2[i], P2[i+2])
  P8[i] = max(P4[i], P4[i+4])
  P9[i] = max(P8[i], P8[i+1])          # max over pairs [i, i+8]
  M19[2i]   = max(P9[i-4], s[2i-9])    # even-parity edge elem (left)
  M19[2i+1] = max(P9[i-4], s[2i+10])   # odd-parity edge elem (right)

Engine split per block: DVE does the hierarchy + T39 + PK + SQDS;
Pool (gpsimd) does the dot and the two peak counts (scalar_tensor_tensor
with accum); Act does the two sums of squares. Both signals ride in one
instruction wherever possible ([128, 2, N] APs).
"""

import sys

if "/opt/trn_rl_repo" not in sys.path:
    sys.path.insert(0, "/opt/trn_rl_repo")

import numpy as np

B, L = 2048, 4096
NCORES = 8
ROWS_PER_CORE = B // NCORES      # 256
NBLK = ROWS_PER_CORE // 128      # 2
PADL = 24                        # left pad (>= 21, even)
PADR = 24
W = PADL + L + PADR              # 4144 (even)
NPAIR = W // 2                   # 2072
ALPHA, BETA = 1.0, 0.5
NEG = -3.0e38                    # stands in for -inf (finite keeps sim happy)

# M19 is produced on t in [PADL-10, PADL+L+10)
M19_LO = PADL - 10               # 14
N_PAR = (PADL + L + 10 - M19_LO + 1) // 2  # outputs per parity (2058)
I_LO = M19_LO // 2               # first even-parity pair index (7)

_CACHE = {}


def _mkap(bass, t, col_off, dims):
    """Custom view of a tile AP `t` ([128, ...]): keep the partition dim,
    replace free dims with explicit [step, count] pairs (element units),
    offset by col_off elements from t's start."""
    part = [list(d) for d in t.ap][0]
    return bass.AP(
        tensor=t.tensor,
        offset=int(t.offset) + int(col_off),
        ap=[part] + [[int(s), int(c)] for s, c in dims],
    )


def _register_custom_ops():
    """Define + self-pin the fused DVE ops, append them to dve_ops.OPS."""
    if "ops" in _CACHE:
        return _CACHE["ops"]
    import concourse.dve_ops as dve_ops
    from concourse.dve_spec import (
        Spec, Src0, Src1, C0, Zero, lower, select, sq, ne,
        _has_src1,
    )
    from concourse.dve_uop import DveOpSpec
    from operator import add as _add

    def _flat2(in0, in1):
        a = np.asarray(in0).reshape(np.asarray(in0).shape[0], -1)
        bb = np.asarray(in1).reshape(np.asarray(in1).shape[0], -1)
        return a, bb

    def _ref_pk(in0, in1, s0, s1, imm2):
        a, bb = _flat2(in0, in1)
        return np.where(a >= bb, a, np.float32(0.0)).astype(np.float32)

    def _ref_cntge(in0, in1, s0, s1, imm2):
        a, bb = _flat2(in0, in1)
        b = (a >= bb).astype(np.float32)
        return b, s0 + b.sum(axis=-1, keepdims=True)

    def _ref_sqds(in0, in1, s0, s1, imm2):
        a, bb = _flat2(in0, in1)
        b = ((a.astype(np.float32) - bb) ** 2).astype(np.float32)
        return b, s0 + b.sum(axis=-1, keepdims=True)

    specs = [
        ("ANT_NMS_PK", Spec(body=select(Src0 >= Src1, Src0, Zero), reference=_ref_pk)),
        (
            "ANT_CNT_GE",
            Spec(
                body=(Src0 >= Src1),
                accum=_add,
                accum_init=C0,
                reference=_ref_cntge,
            ),
        ),
        (
            "ANT_NMS_SQDS",
            Spec(
                body=sq(Src0 - Src1),
                accum=_add,
                accum_init=C0,
                reference=_ref_sqds,
            ),
        ),
    ]

    ops = {}
    for i, (name, spec) in enumerate(specs):
        if any(op.name == name for op in dve_ops.OPS):
            ops[name] = next(op for op in dve_ops.OPS if op.name == name)
            continue
        row = dve_ops._CUSTOM_DVE_ROW_BASE + len(dve_ops.OPS)
        shas = {}
        for ver in ("v3", "v4"):
            r = DveOpSpec(
                name=name, opcode=row, uops=lower(spec, ver=ver),
                rd1_en=_has_src1(spec),
            )
            shas[ver] = r.sha(ver)
        op = dve_ops.DveOp(name, spec, subdim=False, uops_sha=shas)
        dve_ops.OPS.append(op)
        dve_ops.CUSTOM_DVE_SPECS[name] = spec
        ops[name] = op
    dve_ops._SUB_OPCODE_FOR_NAME = {
        op.name: dve_ops._CUSTOM_DVE_ROW_BASE + i for i, op in enumerate(dve_ops.OPS)
    }
    assert max(dve_ops._SUB_OPCODE_FOR_NAME.values()) < 0x20
    _CACHE["ops"] = ops
    return ops


def _build(repeat=1):
    """Build the SPMD program. `repeat` unrolls the whole 2-block body N
    times inside one NEFF (benchmarking only; outputs are just rewritten)."""
    import concourse.bass as bass
    import concourse.bacc as bacc
    import concourse.tile as tile
    import concourse.mybir as mybir
    from contextlib import ExitStack

    ops = _register_custom_ops()
    OP_PK, OP_CNT, OP_SQDS = (
        ops["ANT_NMS_PK"], ops["ANT_CNT_GE"], ops["ANT_NMS_SQDS"],
    )
    from concourse.dve_ops import TENSOR_TENSOR_REDUCE as OP_TTR

    f32 = mybir.dt.float32
    Alu = mybir.AluOpType
    Act = mybir.ActivationFunctionType

    nc = bacc.Bacc("TRN2", target_bir_lowering=False)
    x_in = nc.dram_tensor("x_in", [ROWS_PER_CORE, L], f32, kind="ExternalInput").ap()
    x_ref = nc.dram_tensor("x_ref", [ROWS_PER_CORE, L], f32, kind="ExternalInput").ap()
    out_stats = nc.dram_tensor(
        "stats_out", [NBLK, 128, 6], f32, kind="ExternalOutput"
    ).ap()

    with ExitStack() as ctx:
        tc = ctx.enter_context(tile.TileContext(nc))
        sb = ctx.enter_context(tc.tile_pool(name="sb", bufs=1))
        ps = ctx.enter_context(tc.tile_pool(name="ps", bufs=1, space="PSUM"))

        for rep_b in range(repeat * NBLK):
            b = rep_b % NBLK
            rows = slice(b * 128, (b + 1) * 128)

            # SIG is double-buffered so the next block's loads overlap compute
            SIG = sb.tile([128, 2, W], f32, tag="SIG", bufs=2, name=f"SIG{rep_b}")
            MB = sb.tile([128, 2, W], f32, tag="MB", name=f"MB{rep_b}")
            LVA = sb.tile([128, 2, NPAIR], f32, tag="LVA", name=f"LVA{rep_b}")
            LVB = sb.tile([128, 2, NPAIR], f32, tag="LVB", name=f"LVB{rep_b}")
            T39 = sb.tile([128, 2, L], f32, tag="T39", name=f"T39{rep_b}")
            PK = sb.tile([128, 2, L], f32, tag="PK", name=f"PK{rep_b}")
            STATS = sb.tile([128, 8], f32, tag="STATS", name=f"STATS{rep_b}")
            ACTS = ps.tile([128, L], f32, tag="ACTS", name=f"ACTS{rep_b}")

            sig_h = int(SIG.ap[1][0])  # per-half element strides
            mb_h = int(MB.ap[1][0])
            lva_h = int(LVA.ap[1][0])
            lvb_h = int(LVB.ap[1][0])
            t39_h = int(T39.ap[1][0])

            def both(t, h_stride, off, step, count):
                """AP covering both halves: [128, 2, count] with the given
                inner [step, count] at element offset `off` per half."""
                return _mkap(bass, t, off, [[h_stride, 2], [step, count]])

            # --- load + pad init -------------------------------------------
            nc.sync.dma_start(out=SIG[:, 0, PADL : PADL + L], in_=x_in[rows, :])
            nc.sync.dma_start(out=SIG[:, 1, PADL : PADL + L], in_=x_ref[rows, :])
            nc.gpsimd.memset(SIG[:, :, 0:PADL], NEG)
            nc.gpsimd.memset(SIG[:, :, W - PADR : W], NEG)

            def tmax(out, i0, i1, eng=nc.vector):
                eng.tensor_tensor(out=out, in0=i0, in1=i1, op=Alu.max)

            # --- M19 hierarchy (both halves per instruction) ---------------
            # p[i] = max(s[2i], s[2i+1])
            tmax(
                both(LVA, lva_h, 0, 1, NPAIR),
                both(SIG, sig_h, 0, 2, NPAIR),
                both(SIG, sig_h, 1, 2, NPAIR),
            )
            # P2[i] = max(p[i], p[i+1])
            tmax(
                both(LVB, lvb_h, 0, 1, NPAIR - 1),
                both(LVA, lva_h, 0, 1, NPAIR - 1),
                both(LVA, lva_h, 1, 1, NPAIR - 1),
            )
            # P4[i] = max(P2[i], P2[i+2])
            tmax(
                both(LVA, lva_h, 0, 1, NPAIR - 3),
                both(LVB, lvb_h, 0, 1, NPAIR - 3),
                both(LVB, lvb_h, 2, 1, NPAIR - 3),
            )
            # P8[i] = max(P4[i], P4[i+4])
            tmax(
                both(LVB, lvb_h, 0, 1, NPAIR - 7),
                both(LVA, lva_h, 0, 1, NPAIR - 7),
                both(LVA, lva_h, 4, 1, NPAIR - 7),
            )
            # P9[i] = max(P8[i], P8[i+1])   (max over pairs [i, i+8])
            tmax(
                both(LVA, lva_h, 0, 1, NPAIR - 8),
                both(LVB, lvb_h, 0, 1, NPAIR - 8),
                both(LVB, lvb_h, 1, 1, NPAIR - 8),
            )
            # M19[2i] = max(P9[i-4], s[2i-9]), i in [I_LO, I_LO + N_PAR)
            tmax(
                both(MB, mb_h, 2 * I_LO, 2, N_PAR),
                both(LVA, lva_h, I_LO - 4, 1, N_PAR),
                both(SIG, sig_h, 2 * I_LO - 9, 2, N_PAR),
            )
            # M19[2i+1] = max(P9[i-4], s[2i+10])
            tmax(
                both(MB, mb_h, 2 * I_LO + 1, 2, N_PAR),
                both(LVA, lva_h, I_LO - 4, 1, N_PAR),
                both(SIG, sig_h, 2 * I_LO + 10, 2, N_PAR),
            )
            # T39[j] = max(M19[j-10], M19[j+10])
            tmax(
                both(T39, t39_h, 0, 1, L),
                both(MB, mb_h, PADL - 10, 1, L),
                both(MB, mb_h, PADL + 10, 1, L),
            )
            # pk[j] = s[j] if s[j] >= M19[j] else 0   (d10 peak values)
            nc.vector._custom_dve(
                OP_PK,
                out=both(PK, int(PK.ap[1][0]), 0, 1, L),
                in0=both(SIG, sig_h, PADL, 1, L),
                in1=both(MB, mb_h, PADL, 1, L),
            )
            # reference peaks never occur at j=0 / j=L-1 (local_max padded
            # False there) -> zero PK's edge columns
            nc.vector.memset(
                _mkap(bass, PK, 0, [[int(PK.ap[1][0]), 2], [L - 1, 2]]), 0.0
            )
            # p2p = sum((pk_in - pk_ref)^2) -> stats col 5 (DVE)
            nc.vector._custom_dve(
                OP_SQDS,
                out=_mkap(bass, LVA, 0, [[1, L]]),
                in0=PK[:, 0, 0:L],
                in1=PK[:, 1, 0:L],
                s0=0.0,
                accum_out=STATS[:, 5:6],
            )
            # n20 counts: sum over j in [1, L-2] of (s[j] >= T39[j]) per half
            # -> stats cols 3,4. Edges excluded: the reference's local_max
            # is padded False at j=0 / j=L-1. (Pool can't run elementwise
            # ops on this toolchain, so these ride on DVE as custom ops.)
            for h in range(2):
                nc.vector._custom_dve(
                    OP_CNT,
                    out=_mkap(bass, MB, h * mb_h, [[1, L - 2]]),
                    in0=_mkap(bass, SIG, h * sig_h + PADL + 1, [[1, L - 2]]),
                    in1=_mkap(bass, T39, h * t39_h + 1, [[1, L - 2]]),
                    s0=0.0,
                    accum_out=STATS[:, 3 + h : 4 + h],
                )
            # dot = sum(in*ref) -> stats col 0 (custom-DVE TTR; the stock
            # InstTensorTensorReduce wedges the device on this runtime)
            nc.vector._custom_dve(
                OP_TTR,
                out=_mkap(bass, T39, 0, [[1, L]]),
                in0=_mkap(bass, SIG, 0 * sig_h + PADL, [[1, L]]),
                in1=_mkap(bass, SIG, 1 * sig_h + PADL, [[1, L]]),
                s0=0.0,
                s1=1.0,
                accum_out=STATS[:, 0:1],
            )
            # sums of squares -> stats cols 1,2 (ACT engine)
            for h in range(2):
                nc.scalar.activation(
                    out=ACTS[:, 0:L],
                    in_=_mkap(bass, SIG, h * sig_h + PADL, [[1, L]]),
                    func=Act.Square,
                    accum_out=STATS[:, 1 + h : 2 + h],
                )

            nc.sync.dma_start(out=out_stats[b, :, :], in_=STATS[:, 0:6])

    nc.compile()
    return nc


def _get_nc():
    if "nc" not in _CACHE:
        _CACHE["nc"] = _build()
    return _CACHE["nc"]


def run_device(in_signal, ref_signal):
    """Run the SPMD kernel; returns per-row stats [B, 6] float32."""
    from concourse.bass_utils import run_bass_kernel_spmd

    nc = _get_nc()
    in_maps = []
    for c in range(NCORES):
        r = slice(c * ROWS_PER_CORE, (c + 1) * ROWS_PER_CORE)
        in_maps.append(
            {
                "x_in": np.ascontiguousarray(in_signal[r], dtype=np.float32),
                "x_ref": np.ascontiguousarray(ref_signal[r], dtype=np.float32),
            }
        )
    res = run_bass_kernel_spmd(nc, in_maps, list(range(NCORES))).results
    stats = np.concatenate(
        [np.asarray(res[c]["stats_out"]).reshape(ROWS_PER_CORE, 6) for c in range(NCORES)],
        axis=0,
    )
    return stats


def finalize(stats):
    """Host combine of per-row stats -> [4] f32 output."""
    dot = stats[:, 0].astype(np.float64)
    na2 = stats[:, 1].astype(np.float64)
    nb2 = stats[:, 2].astype(np.float64)
    n_in = stats[:, 3]
    n_ref = stats[:, 4]
    p2p_sum = stats[:, 5].astype(np.float64)

    sqsum = na2 + nb2 - 2.0 * dot
    mse_i = sqsum / L
    mse_loss = sqsum.sum() / (B * L)
    cosine = (dot / np.sqrt(na2 * nb2)).mean()
    p2p_i = p2p_sum / L
    p2p_loss = p2p_i.sum()
    custom = np.where(n_in != n_ref, mse_i * ALPHA, p2p_i * BETA).sum()
    total = mse_loss + custom
    return np.array([total, cosine, p2p_loss, mse_loss], dtype=np.float32)


def kernel(in_signal, ref_signal):
    stats = run_device(np.asarray(in_signal), np.asarray(ref_signal))
    return finalize(stats)
